# revision 2
# baseline (speedup 1.0000x reference)
"""AttentionGRUCell fused kernel v3 for 8 Trainium2 NeuronCores.

Data-parallel over the batch dim (2048 rows/core).  Inputs x,h,a are
cast to bf16 and concatenated on the host into one [b_l, 4096] DRAM
tensor.  Every k-major matmul operand tile is produced by a DMA-XBAR
transpose straight from DRAM; intermediates rh and s spill to DRAM in
row layout (bf16) and are re-read through the XBAR.

The four gate phases (z, r, s~, t) run as ONE flat software pipeline
over 4*n_bt tiles: operand transposes/loads are issued with a global
2-tile lookahead that crosses phase boundaries, so the PE never waits
on the strict-FIFO ACT queue at a phase switch.  Each phase has
exactly one gate's weight set resident ([128,8,1024] x2 +
[128,16,1024] bf16 = 64 KiB/partition), loaded on the sync queue one
phase ahead.

Queue policy (measured): sync (HWDGE) = weights + small loads + fp32
output stores; scalar (HWDGE) = XBAR transposes only; gpsimd (SWDGE)
= small spill stores and the phase-r h load only.

  z: xhaT <- T(xhab) ; 64 MM -> sigmoid -> z16 spill
  r: xhaT <- T(xhab) ; 64 MM -> sigmoid -> rh = r*h spill
  s: lT = [T(x)|T(rh)|T(a)] ; 64 MM -> tanh -> s = h + z*(s~-h)
     -> store s (f32) + s16 spill
  t: lT = [T(x)|T(a)|T(s16)] ; 64 MM -> relu -> store t
"""

import sys

if "/opt/trn_rl_repo" not in sys.path:
    sys.path.insert(0, "/opt/trn_rl_repo")

import numpy as np

BATCH = 16384
EMB = 1024
HID = 1024
COMB = 2048
N_CORES = 8
B_L = BATCH // N_CORES
P = 128
N_BT_FULL = B_L // P


def _build_nc(n_bt, tweak="", phases="zrst", repeat=1, with_bias=False):
    import concourse.mybir as mybir
    from contextlib import ExitStack
    from concourse import bacc
    from concourse.tile import TileContext

    dt = mybir.dt
    AF = mybir.ActivationFunctionType
    b_l = n_bt * P

    nc = bacc.Bacc("TRN2", target_bir_lowering=False, debug=False,
                   num_devices=N_CORES)

    xhab_d = nc.declare_dram_parameter("xhab", [b_l, EMB + HID + COMB],
                                       dt.bfloat16, isOutput=False)
    wnames = ["Wz", "Uz", "Cz", "Wr", "Ur", "Cr",
              "W", "U", "C", "Vo", "Co", "Uo"]
    wshapes = {n: ([COMB, HID] if n.startswith("C") else [EMB, HID])
               for n in wnames}
    wd = {n: nc.declare_dram_parameter(n, wshapes[n], dt.bfloat16,
                                       isOutput=False)
          for n in wnames}
    bias_d = {}
    if with_bias:
        for g in ("z", "r", "s", "t"):
            bias_d[g] = nc.declare_dram_parameter(
                f"bias_{g}", [P, HID], dt.float32, isOutput=False)
    s_out = nc.declare_dram_parameter("s", [b_l, HID], dt.float32,
                                      isOutput=True)
    t_out = nc.declare_dram_parameter("t", [b_l, HID], dt.float32,
                                      isOutput=True)

    rh_d = nc.dram_tensor("rh_spill", [n_bt, P, HID], dt.bfloat16)
    s16_d = nc.dram_tensor("s16_spill", [n_bt, P, HID], dt.bfloat16)
    z_d = nc.dram_tensor("z_spill", [n_bt, P, HID], dt.float16)

    lookahead = 2
    if "la3" in tweak:
        lookahead = 3
    lt_bufs = lookahead + 1

    with TileContext(nc) as tc, ExitStack() as top:
        wp = top.enter_context(tc.tile_pool(name="w", bufs=2))
        tp = top.enter_context(tc.tile_pool(name="tp", bufs=lt_bufs))
        ld = top.enter_context(tc.tile_pool(name="ld", bufs=lt_bufs))
        ev = top.enter_context(tc.tile_pool(
            name="ev", bufs=(3 if "ev3" in tweak else 2)))
        ps = top.enter_context(tc.tile_pool(
            name="ps", bufs=(4 if "ps4" in tweak else 3), space="PSUM"))
        bp = top.enter_context(tc.tile_pool(name="bp", bufs=1))

        wload_count = [0]

        def load_w(*mats):
            # cold start: the very first gate set splits its w16 onto
            # gpsimd so the PE isn't starved behind one 8 MiB queue.
            first = wload_count[0] == 0
            wload_count[0] += 1
            out = []
            for wdram in mats:
                nkb = wdram.shape[0] // P
                wt = wp.tile([P, nkb, HID], dt.bfloat16, tag=f"w{nkb}",
                             bufs=(4 if nkb == 8 else 2))
                eng = nc.sync
                if first and nkb == 16 and "nocs" not in tweak:
                    eng = nc.gpsimd
                elif nkb == 16 and "w16sc" in tweak:
                    eng = nc.scalar
                for kb in range(0, nkb, 2):
                    eng.dma_start(
                        wt[:, kb:kb + 2, :],
                        wdram[kb * P:(kb + 2) * P, :].rearrange(
                            "(kb p) n -> p kb n", p=P))
                out.append(wt)
            return out

        biases = {}

        def load_bias(g):
            if not with_bias:
                return None
            if g not in biases:
                b = bp.tile([P, HID], dt.float32, tag=f"b{g}")
                nc.sync.dma_start(b[:], bias_d[g][:])
                biases[g] = b
            return biases[g]

        def mm_gate(pp, lhs_of_kb, wts):
            tot = sum(wt.shape[1] for wt in wts)
            kb0 = 0
            for wt in wts:
                nkb = wt.shape[1]
                for j in range(nkb):
                    kb = kb0 + j
                    st = kb == 0
                    sp = kb == tot - 1
                    lhsT = lhs_of_kb(kb)
                    nc.tensor.matmul(pp[0][:], lhsT, wt[:, j, 0:512],
                                     start=st, stop=sp)
                    nc.tensor.matmul(pp[1][:], lhsT, wt[:, j, 512:1024],
                                     start=st, stop=sp)
                kb0 += nkb

        def psum_pair():
            p0 = ps.tile([P, 512], dt.float32, tag="p0")
            p1 = ps.tile([P, 512], dt.float32, tag="p1")
            return p0, p1

        def badd(pp, g):
            b = load_bias(g)
            if b is not None:
                nc.vector.tensor_add(pp[0][:], pp[0][:], b[:, 0:512])
                nc.vector.tensor_add(pp[1][:], pp[1][:], b[:, 512:1024])

        # ----------------------------------------------------- phase z
        def w_z():
            load_bias("z")
            return load_w(wd["Wz"], wd["Uz"], wd["Cz"])

        def tin_z(bt):
            r0 = bt * P
            xhaT = tp.tile([P, 32, P], dt.bfloat16, tag="lT")
            nc.scalar.dma_start(xhaT[:], xhab_d[r0:r0 + P, :],
                                transpose=True)
            return (xhaT,)

        def body_z(bt, ins, w):
            (xhaT,) = ins
            pp = psum_pair()
            mm_gate(pp, lambda kb: xhaT[:, kb, :], w)
            badd(pp, "z")
            z16 = ev.tile([P, HID], dt.float16, tag="z16")
            nc.scalar.activation(z16[:, 0:512], pp[0][:], AF.Sigmoid)
            nc.scalar.activation(z16[:, 512:1024], pp[1][:], AF.Sigmoid)
            spq = nc.sync if "spsy" in tweak else nc.gpsimd
            spq.dma_start(z_d[bt], z16[:])

        # ----------------------------------------------------- phase r
        def w_r():
            load_bias("r")
            return load_w(wd["Wr"], wd["Ur"], wd["Cr"])

        def tin_r(bt):
            r0 = bt * P
            xhaT = tp.tile([P, 32, P], dt.bfloat16, tag="lT")
            nc.scalar.dma_start(xhaT[:], xhab_d[r0:r0 + P, :],
                                transpose=True)
            hb = ld.tile([P, HID], dt.bfloat16, tag="hb")
            nc.gpsimd.dma_start(hb[:], xhab_d[r0:r0 + P, EMB:EMB + HID])
            return xhaT, hb

        def body_r(bt, ins, w):
            xhaT, hb = ins
            pp = psum_pair()
            mm_gate(pp, lambda kb: xhaT[:, kb, :], w)
            badd(pp, "r")
            rhb = ev.tile([P, HID], dt.bfloat16, tag="rhb")
            for half, px in enumerate(pp):
                sl = slice(half * 512, half * 512 + 512)
                nc.scalar.activation(px[:], px[:], AF.Sigmoid)
                nc.vector.tensor_mul(rhb[:, sl], px[:], hb[:, sl])
            spq = nc.sync if "spsy" in tweak else nc.gpsimd
            spq.dma_start(rh_d[bt], rhb[:])

        # ----------------------------------------------------- phase s
        def w_s():
            load_bias("s")
            return load_w(wd["W"], wd["U"], wd["C"])

        def tin_s(bt):
            r0 = bt * P
            lT = tp.tile([P, 32, P], dt.bfloat16, tag="lT")
            nc.scalar.dma_start(lT[:, 0:8, :], xhab_d[r0:r0 + P, 0:EMB],
                                transpose=True)
            nc.scalar.dma_start(lT[:, 8:16, :], rh_d[bt], transpose=True)
            nc.scalar.dma_start(lT[:, 16:32, :],
                                xhab_d[r0:r0 + P, EMB + HID:],
                                transpose=True)
            z16 = ld.tile([P, HID], dt.float16, tag="z16")
            nc.sync.dma_start(z16[:], z_d[bt])
            hf = ld.tile([P, HID], dt.bfloat16, tag="hf")
            nc.sync.dma_start(hf[:], xhab_d[r0:r0 + P, EMB:EMB + HID])
            return lT, z16, hf

        def body_s(bt, ins, w):
            lT, z16, hf = ins
            r0 = bt * P
            pp = psum_pair()
            mm_gate(pp, lambda kb: lT[:, kb, :], w)
            badd(pp, "s")
            stil = ev.tile([P, HID], dt.float32, tag="stil")
            nc.scalar.activation(stil[:, 0:512], pp[0][:], AF.Tanh)
            nc.scalar.activation(stil[:, 512:1024], pp[1][:], AF.Tanh)
            nc.vector.tensor_sub(stil[:], stil[:], hf[:])
            nc.vector.tensor_mul(stil[:], z16[:], stil[:])
            nc.vector.tensor_add(stil[:], hf[:], stil[:])
            nc.sync.dma_start(s_out[r0:r0 + P, :], stil[:])
            sb2 = ev.tile([P, HID], dt.bfloat16, tag="sb2")
            nc.vector.tensor_copy(sb2[:], stil[:])
            spq = nc.sync if "spsy" in tweak else nc.gpsimd
            spq.dma_start(s16_d[bt], sb2[:])

        # ----------------------------------------------------- phase t
        def w_t():
            load_bias("t")
            return load_w(wd["Vo"], wd["Co"], wd["Uo"])

        def tin_t(bt):
            r0 = bt * P
            lT = tp.tile([P, 32, P], dt.bfloat16, tag="lT")
            nc.scalar.dma_start(lT[:, 0:8, :], xhab_d[r0:r0 + P, 0:EMB],
                                transpose=True)
            nc.scalar.dma_start(lT[:, 8:24, :],
                                xhab_d[r0:r0 + P, EMB + HID:],
                                transpose=True)
            nc.scalar.dma_start(lT[:, 24:32, :], s16_d[bt], transpose=True)
            return (lT,)

        def body_t(bt, ins, w):
            (lT,) = ins
            r0 = bt * P
            pp = psum_pair()
            mm_gate(pp, lambda kb: lT[:, kb, :], w)
            badd(pp, "t")
            tf = ev.tile([P, HID], dt.float32, tag="tf")
            nc.scalar.activation(tf[:, 0:512], pp[0][:], AF.Relu)
            nc.scalar.activation(tf[:, 512:1024], pp[1][:], AF.Relu)
            nc.sync.dma_start(t_out[r0:r0 + P, :], tf[:])

        all_specs = {"z": (w_z, tin_z, body_z), "r": (w_r, tin_r, body_r),
                     "s": (w_s, tin_s, body_s), "t": (w_t, tin_t, body_t)}
        specs = []
        for _ in range(repeat):
            for ch in "zrst":
                if ch in phases:
                    specs.append(all_specs[ch])

        wtiles = {}

        def ensure_w(i):
            if 0 <= i < len(specs) and i not in wtiles:
                wtiles[i] = specs[i][0]()

        ensure_w(0)
        ensure_w(1)
        G = len(specs) * n_bt
        pend = {}
        nxt = 0
        for g in range(G):
            while nxt <= g + lookahead and nxt < G:
                p_, bt_ = divmod(nxt, n_bt)
                pend[nxt] = specs[p_][1](bt_)
                nxt += 1
            p_, bt_ = divmod(g, n_bt)
            specs[p_][2](bt_, pend.pop(g), wtiles[p_])
            if bt_ == n_bt - 1:
                ensure_w(p_ + 2)

    nc.compile()
    return nc


_CACHE = {}


def _get_exec(n_bt, tweak="", phases="zrst", repeat=1, with_bias=False):
    key = (n_bt, tweak, phases, repeat, with_bias)
    if key in _CACHE:
        return _CACHE[key]

    import jax
    import concourse.mybir as mybir
    from concourse import bass2jax
    from jax.sharding import Mesh, PartitionSpec
    from jax.experimental.shard_map import shard_map

    bass2jax.install_neuronx_cc_hook()
    nc = _build_nc(n_bt, tweak, phases, repeat, with_bias)

    partition_name = (nc.partition_id_tensor.name
                      if nc.partition_id_tensor else None)
    in_names = []
    out_names = []
    out_avals = []
    zero_outs = []
    for alloc in nc.m.functions[0].allocations:
        if not isinstance(alloc, mybir.MemoryLocationSet):
            continue
        name = alloc.memorylocations[0].name
        if alloc.kind == "ExternalInput":
            if name != partition_name:
                in_names.append(name)
        elif alloc.kind == "ExternalOutput":
            out_names.append(name)
            shape = tuple(alloc.tensor_shape)
            dtype = mybir.dt.np(alloc.dtype)
            out_avals.append(jax.core.ShapedArray(shape, dtype))
            zero_outs.append(np.zeros(shape, dtype))
    n_params = len(in_names)
    all_in_names = in_names + out_names
    if partition_name is not None:
        all_in_names = all_in_names + [partition_name]

    def _body(*args):
        operands = list(args)
        if partition_name is not None:
            operands.append(bass2jax.partition_id_tensor())
        outs = bass2jax._bass_exec_p.bind(
            *operands,
            out_avals=tuple(out_avals),
            in_names=tuple(all_in_names),
            out_names=tuple(out_names),
            lowering_input_output_aliases=(),
            sim_require_finite=True,
            sim_require_nnan=True,
            nc=nc,
        )
        return tuple(outs)

    devices = jax.devices()[:N_CORES]
    mesh = Mesh(np.asarray(devices), ("core",))
    n_outs = len(out_names)
    sharded = jax.jit(
        shard_map(
            _body, mesh=mesh,
            in_specs=(PartitionSpec("core"),) * (n_params + n_outs),
            out_specs=(PartitionSpec("core"),) * n_outs,
            check_rep=False,
        ),
        keep_unused=True,
    )
    entry = {
        "nc": nc,
        "sharded": sharded,
        "in_names": in_names,
        "out_names": out_names,
        "zero_outs": zero_outs,
        "mesh": mesh,
    }
    _CACHE[key] = entry
    return entry


def _prepare_in_arrays(entry, inputs, bias_rows=None):
    import ml_dtypes
    bf16 = ml_dtypes.bfloat16
    xhab = np.concatenate(
        [np.asarray(inputs["in_word"], np.float32).astype(bf16),
         np.asarray(inputs["last_hid_state"], np.float32).astype(bf16),
         np.asarray(inputs["attended_state"], np.float32).astype(bf16)],
        axis=1)
    arrs = []
    for name in entry["in_names"]:
        if name == "xhab":
            arrs.append(np.ascontiguousarray(xhab))
        elif name.startswith("bias_"):
            g = name.split("_")[1]
            row = np.broadcast_to(np.asarray(bias_rows[g], np.float32),
                                  (P, HID))
            arrs.append(np.ascontiguousarray(np.tile(row, (N_CORES, 1))))
        else:
            w = np.asarray(inputs[name], dtype=np.float32).astype(bf16)
            arrs.append(np.ascontiguousarray(np.tile(w, (N_CORES, 1))))
    return arrs


def kernel(in_word, last_hid_state, attended_state,
           W, bw, Wz, bwz, Wr, bwr,
           U, bu, Uz, buz, Ur, bur,
           C, bc, Cz, bcz, Cr, bcr,
           Uo, buo, Vo, bvo, Co, bco):
    inputs = dict(in_word=np.asarray(in_word),
                  last_hid_state=np.asarray(last_hid_state),
                  attended_state=np.asarray(attended_state),
                  W=W, Wz=Wz, Wr=Wr, U=U, Uz=Uz, Ur=Ur,
                  C=C, Cz=Cz, Cr=Cr, Uo=Uo, Vo=Vo, Co=Co)
    bias_rows = {
        "z": np.asarray(bwz) + np.asarray(buz) + np.asarray(bcz),
        "r": np.asarray(bwr) + np.asarray(bur) + np.asarray(bcr),
        "s": np.asarray(bw) + np.asarray(bu) + np.asarray(bc),
        "t": np.asarray(buo) + np.asarray(bvo) + np.asarray(bco),
    }
    with_bias = bool(any(np.any(np.asarray(v) != 0)
                         for v in bias_rows.values()))

    entry = _get_exec(N_BT_FULL, with_bias=with_bias)
    arrs = _prepare_in_arrays(entry, inputs, bias_rows)
    zeros = [np.zeros((N_CORES * z.shape[0], *z.shape[1:]), z.dtype)
             for z in entry["zero_outs"]]
    outs = entry["sharded"](*arrs, *zeros)
    res = {name: np.asarray(outs[i])
           for i, name in enumerate(entry["out_names"])}
    return (res["s"], res["t"])


# revision 4
# speedup vs baseline: 1.1628x; 1.1628x over previous
"""AttentionGRUCell fused kernel v3 for 8 Trainium2 NeuronCores.

Data-parallel over the batch dim (2048 rows/core).  Inputs x,h,a are
cast to bf16 and concatenated on the host into one [b_l, 4096] DRAM
tensor.  Every k-major matmul operand tile is produced by a DMA-XBAR
transpose straight from DRAM; intermediates rh and s spill to DRAM in
row layout (bf16) and are re-read through the XBAR.

The four gate phases (z, r, s~, t) run as ONE flat software pipeline
over 4*n_bt tiles: operand transposes/loads are issued with a global
2-tile lookahead that crosses phase boundaries, so the PE never waits
on the strict-FIFO ACT queue at a phase switch.  Each phase has
exactly one gate's weight set resident ([128,8,1024] x2 +
[128,16,1024] bf16 = 64 KiB/partition), loaded on the sync queue one
phase ahead.

Queue policy (measured): sync (HWDGE) = weights + small loads + fp32
output stores; scalar (HWDGE) = XBAR transposes only; gpsimd (SWDGE)
= small spill stores and the phase-r h load only.

  z: xhaT <- T(xhab) ; 64 MM -> sigmoid -> z16 spill
  r: xhaT <- T(xhab) ; 64 MM -> sigmoid -> rh = r*h spill
  s: lT = [T(x)|T(rh)|T(a)] ; 64 MM -> tanh -> s = h + z*(s~-h)
     -> store s (f32) + s16 spill
  t: lT = [T(x)|T(a)|T(s16)] ; 64 MM -> relu -> store t
"""

import sys

if "/opt/trn_rl_repo" not in sys.path:
    sys.path.insert(0, "/opt/trn_rl_repo")

import numpy as np

BATCH = 16384
EMB = 1024
HID = 1024
COMB = 2048
N_CORES = 8
B_L = BATCH // N_CORES
P = 128
N_BT_FULL = B_L // P


def _build_nc(n_bt, tweak="", phases="zrst", repeat=1, with_bias=False):
    import concourse.mybir as mybir
    from contextlib import ExitStack
    from concourse import bacc
    from concourse.tile import TileContext

    dt = mybir.dt
    AF = mybir.ActivationFunctionType
    b_l = n_bt * P

    nc = bacc.Bacc("TRN2", target_bir_lowering=False, debug=False,
                   num_devices=N_CORES)

    xhab_d = nc.declare_dram_parameter("xhab", [b_l, EMB + HID + COMB],
                                       dt.bfloat16, isOutput=False)
    wnames = ["Wz", "Uz", "Cz", "Wr", "Ur", "Cr",
              "W", "U", "C", "Vo", "Co", "Uo"]
    wshapes = {n: ([COMB, HID] if n.startswith("C") else [EMB, HID])
               for n in wnames}
    wd = {n: nc.declare_dram_parameter(n, wshapes[n], dt.bfloat16,
                                       isOutput=False)
          for n in wnames}
    bias_d = {}
    if with_bias:
        for g in ("z", "r", "s", "t"):
            bias_d[g] = nc.declare_dram_parameter(
                f"bias_{g}", [P, HID], dt.float16, isOutput=False)
    s_out = nc.declare_dram_parameter("s", [b_l, HID], dt.float32,
                                      isOutput=True)
    t_out = nc.declare_dram_parameter("t", [b_l, HID], dt.float32,
                                      isOutput=True)

    rh_d = nc.dram_tensor("rh_spill", [n_bt, P, HID], dt.bfloat16)
    s16_d = nc.dram_tensor("s16_spill", [n_bt, P, HID], dt.bfloat16)
    z_d = nc.dram_tensor("z_spill", [n_bt, P, HID], dt.float16)

    lookahead = 2
    if "la3" in tweak:
        lookahead = 3
    lt_bufs = lookahead + 1

    with TileContext(nc) as tc, ExitStack() as top:
        wp = top.enter_context(tc.tile_pool(name="w", bufs=2))
        tp = top.enter_context(tc.tile_pool(name="tp", bufs=lt_bufs))
        ld = top.enter_context(tc.tile_pool(name="ld", bufs=lt_bufs))
        ev = top.enter_context(tc.tile_pool(
            name="ev", bufs=(3 if "ev3" in tweak else 2)))
        ps = top.enter_context(tc.tile_pool(
            name="ps", bufs=(4 if "ps4" in tweak else 3), space="PSUM"))
        bp = top.enter_context(tc.tile_pool(name="bp", bufs=1))

        wload_count = [0]

        def load_w(*mats):
            # cold start: the very first gate set splits its w16 onto
            # gpsimd so the PE isn't starved behind one 8 MiB queue.
            first = wload_count[0] == 0
            wload_count[0] += 1
            out = []
            for wdram in mats:
                nkb = wdram.shape[0] // P
                wt = wp.tile([P, nkb, HID], dt.bfloat16, tag=f"w{nkb}",
                             bufs=(4 if nkb == 8 else 2))
                eng = nc.sync
                if first and nkb == 16 and "nocs" not in tweak:
                    eng = nc.gpsimd
                elif nkb == 16 and "w16sc" in tweak:
                    eng = nc.scalar
                for kb in range(0, nkb, 2):
                    eng.dma_start(
                        wt[:, kb:kb + 2, :],
                        wdram[kb * P:(kb + 2) * P, :].rearrange(
                            "(kb p) n -> p kb n", p=P))
                out.append(wt)
            return out

        biases = {}

        def load_bias(g):
            if not with_bias:
                return None
            if g not in biases:
                b = bp.tile([P, HID], dt.float16, tag=f"b{g}")
                nc.sync.dma_start(b[:], bias_d[g][:])
                biases[g] = b
            return biases[g]

        def mm_gate(pp, lhs_of_kb, wts):
            tot = sum(wt.shape[1] for wt in wts)
            kb0 = 0
            for wt in wts:
                nkb = wt.shape[1]
                for j in range(nkb):
                    kb = kb0 + j
                    st = kb == 0
                    sp = kb == tot - 1
                    lhsT = lhs_of_kb(kb)
                    nc.tensor.matmul(pp[0][:], lhsT, wt[:, j, 0:512],
                                     start=st, stop=sp)
                    nc.tensor.matmul(pp[1][:], lhsT, wt[:, j, 512:1024],
                                     start=st, stop=sp)
                kb0 += nkb

        def psum_pair():
            p0 = ps.tile([P, 512], dt.float32, tag="p0")
            p1 = ps.tile([P, 512], dt.float32, tag="p1")
            return p0, p1

        def badd(pp, g):
            b = load_bias(g)
            if b is not None:
                nc.vector.tensor_add(pp[0][:], pp[0][:], b[:, 0:512])
                nc.vector.tensor_add(pp[1][:], pp[1][:], b[:, 512:1024])

        # ----------------------------------------------------- phase z
        def w_z():
            load_bias("z")
            return load_w(wd["Wz"], wd["Uz"], wd["Cz"])

        def tin_z(bt):
            r0 = bt * P
            xhaT = tp.tile([P, 32, P], dt.bfloat16, tag="lT")
            nc.scalar.dma_start(xhaT[:], xhab_d[r0:r0 + P, :],
                                transpose=True)
            return (xhaT,)

        def body_z(bt, ins, w):
            (xhaT,) = ins
            pp = psum_pair()
            mm_gate(pp, lambda kb: xhaT[:, kb, :], w)
            badd(pp, "z")
            z16 = ev.tile([P, HID], dt.float16, tag="z16")
            nc.scalar.activation(z16[:, 0:512], pp[0][:], AF.Sigmoid)
            nc.scalar.activation(z16[:, 512:1024], pp[1][:], AF.Sigmoid)
            spq = nc.sync if "spsy" in tweak else nc.gpsimd
            spq.dma_start(z_d[bt], z16[:])

        # ----------------------------------------------------- phase r
        def w_r():
            load_bias("r")
            return load_w(wd["Wr"], wd["Ur"], wd["Cr"])

        def tin_r(bt):
            r0 = bt * P
            xhaT = tp.tile([P, 32, P], dt.bfloat16, tag="lT")
            nc.scalar.dma_start(xhaT[:], xhab_d[r0:r0 + P, :],
                                transpose=True)
            hb = ld.tile([P, HID], dt.bfloat16, tag="hb")
            nc.gpsimd.dma_start(hb[:], xhab_d[r0:r0 + P, EMB:EMB + HID])
            return xhaT, hb

        def body_r(bt, ins, w):
            xhaT, hb = ins
            pp = psum_pair()
            mm_gate(pp, lambda kb: xhaT[:, kb, :], w)
            badd(pp, "r")
            rhb = ev.tile([P, HID], dt.bfloat16, tag="rhb")
            for half, px in enumerate(pp):
                sl = slice(half * 512, half * 512 + 512)
                nc.scalar.activation(px[:], px[:], AF.Sigmoid)
                nc.vector.tensor_mul(rhb[:, sl], px[:], hb[:, sl])
            spq = nc.sync if "spsy" in tweak else nc.gpsimd
            spq.dma_start(rh_d[bt], rhb[:])

        # ----------------------------------------------------- phase s
        def w_s():
            load_bias("s")
            return load_w(wd["W"], wd["U"], wd["C"])

        def tin_s(bt):
            r0 = bt * P
            lT = tp.tile([P, 32, P], dt.bfloat16, tag="lT")
            nc.scalar.dma_start(lT[:, 0:8, :], xhab_d[r0:r0 + P, 0:EMB],
                                transpose=True)
            nc.scalar.dma_start(lT[:, 8:16, :], rh_d[bt], transpose=True)
            nc.scalar.dma_start(lT[:, 16:32, :],
                                xhab_d[r0:r0 + P, EMB + HID:],
                                transpose=True)
            z16 = ld.tile([P, HID], dt.float16, tag="z16")
            nc.sync.dma_start(z16[:], z_d[bt])
            hf = ld.tile([P, HID], dt.bfloat16, tag="hf")
            nc.sync.dma_start(hf[:], xhab_d[r0:r0 + P, EMB:EMB + HID])
            return lT, z16, hf

        def body_s(bt, ins, w):
            lT, z16, hf = ins
            r0 = bt * P
            pp = psum_pair()
            mm_gate(pp, lambda kb: lT[:, kb, :], w)
            badd(pp, "s")
            stil = ev.tile([P, HID], dt.float32, tag="stil")
            nc.scalar.activation(stil[:, 0:512], pp[0][:], AF.Tanh)
            nc.scalar.activation(stil[:, 512:1024], pp[1][:], AF.Tanh)
            nc.vector.tensor_sub(stil[:], stil[:], hf[:])
            nc.vector.tensor_mul(stil[:], z16[:], stil[:])
            nc.vector.tensor_add(stil[:], hf[:], stil[:])
            nc.sync.dma_start(s_out[r0:r0 + P, :], stil[:])
            sb2 = ev.tile([P, HID], dt.bfloat16, tag="sb2")
            nc.vector.tensor_copy(sb2[:], stil[:])
            spq = nc.sync if "spsy" in tweak else nc.gpsimd
            spq.dma_start(s16_d[bt], sb2[:])

        # ----------------------------------------------------- phase t
        def w_t():
            load_bias("t")
            return load_w(wd["Vo"], wd["Co"], wd["Uo"])

        def tin_t(bt):
            r0 = bt * P
            lT = tp.tile([P, 32, P], dt.bfloat16, tag="lT")
            nc.scalar.dma_start(lT[:, 0:8, :], xhab_d[r0:r0 + P, 0:EMB],
                                transpose=True)
            nc.scalar.dma_start(lT[:, 8:24, :],
                                xhab_d[r0:r0 + P, EMB + HID:],
                                transpose=True)
            nc.scalar.dma_start(lT[:, 24:32, :], s16_d[bt], transpose=True)
            return (lT,)

        def body_t(bt, ins, w):
            (lT,) = ins
            r0 = bt * P
            pp = psum_pair()
            mm_gate(pp, lambda kb: lT[:, kb, :], w)
            badd(pp, "t")
            tf = ev.tile([P, HID], dt.float32, tag="tf")
            nc.scalar.activation(tf[:, 0:512], pp[0][:], AF.Relu)
            nc.scalar.activation(tf[:, 512:1024], pp[1][:], AF.Relu)
            nc.sync.dma_start(t_out[r0:r0 + P, :], tf[:])

        all_specs = {"z": (w_z, tin_z, body_z), "r": (w_r, tin_r, body_r),
                     "s": (w_s, tin_s, body_s), "t": (w_t, tin_t, body_t)}
        specs = []
        for _ in range(repeat):
            for ch in "zrst":
                if ch in phases:
                    specs.append(all_specs[ch])

        wtiles = {}

        def ensure_w(i):
            if 0 <= i < len(specs) and i not in wtiles:
                wtiles[i] = specs[i][0]()

        ensure_w(0)
        ensure_w(1)
        G = len(specs) * n_bt
        pend = {}
        nxt = 0
        for g in range(G):
            while nxt <= g + lookahead and nxt < G:
                p_, bt_ = divmod(nxt, n_bt)
                pend[nxt] = specs[p_][1](bt_)
                nxt += 1
            p_, bt_ = divmod(g, n_bt)
            specs[p_][2](bt_, pend.pop(g), wtiles[p_])
            if bt_ == n_bt - 1:
                ensure_w(p_ + 2)

    nc.compile()
    return nc


_CACHE = {}


def _get_exec(n_bt, tweak="", phases="zrst", repeat=1, with_bias=False):
    key = (n_bt, tweak, phases, repeat, with_bias)
    if key in _CACHE:
        return _CACHE[key]

    import jax
    import concourse.mybir as mybir
    from concourse import bass2jax
    from jax.sharding import Mesh, PartitionSpec
    from jax.experimental.shard_map import shard_map

    bass2jax.install_neuronx_cc_hook()
    nc = _build_nc(n_bt, tweak, phases, repeat, with_bias)

    partition_name = (nc.partition_id_tensor.name
                      if nc.partition_id_tensor else None)
    in_names = []
    out_names = []
    out_avals = []
    zero_outs = []
    for alloc in nc.m.functions[0].allocations:
        if not isinstance(alloc, mybir.MemoryLocationSet):
            continue
        name = alloc.memorylocations[0].name
        if alloc.kind == "ExternalInput":
            if name != partition_name:
                in_names.append(name)
        elif alloc.kind == "ExternalOutput":
            out_names.append(name)
            shape = tuple(alloc.tensor_shape)
            dtype = mybir.dt.np(alloc.dtype)
            out_avals.append(jax.core.ShapedArray(shape, dtype))
            zero_outs.append(np.zeros(shape, dtype))
    n_params = len(in_names)
    all_in_names = in_names + out_names
    if partition_name is not None:
        all_in_names = all_in_names + [partition_name]

    def _body(*args):
        operands = list(args)
        if partition_name is not None:
            operands.append(bass2jax.partition_id_tensor())
        outs = bass2jax._bass_exec_p.bind(
            *operands,
            out_avals=tuple(out_avals),
            in_names=tuple(all_in_names),
            out_names=tuple(out_names),
            lowering_input_output_aliases=(),
            sim_require_finite=True,
            sim_require_nnan=True,
            nc=nc,
        )
        return tuple(outs)

    devices = jax.devices()[:N_CORES]
    mesh = Mesh(np.asarray(devices), ("core",))
    n_outs = len(out_names)
    sharded = jax.jit(
        shard_map(
            _body, mesh=mesh,
            in_specs=(PartitionSpec("core"),) * (n_params + n_outs),
            out_specs=(PartitionSpec("core"),) * n_outs,
            check_rep=False,
        ),
        keep_unused=True,
    )
    entry = {
        "nc": nc,
        "sharded": sharded,
        "in_names": in_names,
        "out_names": out_names,
        "zero_outs": zero_outs,
        "mesh": mesh,
    }
    _CACHE[key] = entry
    return entry


def _prepare_in_arrays(entry, inputs, bias_rows=None):
    import ml_dtypes
    bf16 = ml_dtypes.bfloat16
    xhab = np.concatenate(
        [np.asarray(inputs["in_word"], np.float32).astype(bf16),
         np.asarray(inputs["last_hid_state"], np.float32).astype(bf16),
         np.asarray(inputs["attended_state"], np.float32).astype(bf16)],
        axis=1)
    arrs = []
    for name in entry["in_names"]:
        if name == "xhab":
            arrs.append(np.ascontiguousarray(xhab))
        elif name.startswith("bias_"):
            g = name.split("_")[1]
            row = np.broadcast_to(np.asarray(bias_rows[g], np.float16),
                                  (P, HID))
            arrs.append(np.ascontiguousarray(np.tile(row, (N_CORES, 1))))
        else:
            w = np.asarray(inputs[name], dtype=np.float32).astype(bf16)
            arrs.append(np.ascontiguousarray(np.tile(w, (N_CORES, 1))))
    return arrs


def kernel(in_word, last_hid_state, attended_state,
           W, bw, Wz, bwz, Wr, bwr,
           U, bu, Uz, buz, Ur, bur,
           C, bc, Cz, bcz, Cr, bcr,
           Uo, buo, Vo, bvo, Co, bco):
    inputs = dict(in_word=np.asarray(in_word),
                  last_hid_state=np.asarray(last_hid_state),
                  attended_state=np.asarray(attended_state),
                  W=W, Wz=Wz, Wr=Wr, U=U, Uz=Uz, Ur=Ur,
                  C=C, Cz=Cz, Cr=Cr, Uo=Uo, Vo=Vo, Co=Co)
    bias_rows = {
        "z": np.asarray(bwz) + np.asarray(buz) + np.asarray(bcz),
        "r": np.asarray(bwr) + np.asarray(bur) + np.asarray(bcr),
        "s": np.asarray(bw) + np.asarray(bu) + np.asarray(bc),
        "t": np.asarray(buo) + np.asarray(bvo) + np.asarray(bco),
    }
    with_bias = bool(any(np.any(np.asarray(v) != 0)
                         for v in bias_rows.values()))

    entry = _get_exec(N_BT_FULL, with_bias=with_bias)
    arrs = _prepare_in_arrays(entry, inputs, bias_rows)
    zeros = [np.zeros((N_CORES * z.shape[0], *z.shape[1:]), z.dtype)
             for z in entry["zero_outs"]]
    outs = entry["sharded"](*arrs, *zeros)
    res = {name: np.asarray(outs[i])
           for i, name in enumerate(entry["out_names"])}
    return (res["s"], res["t"])
